# revision 2
# baseline (speedup 1.0000x reference)
"""Causal self-attention (B=8, T=1024, C=1024, H=16) on 8 trn2 NeuronCores.

Data-parallel over batch: each core computes one batch element. All matmul
inputs are bf16 (host-cast); accumulation is fp32 in PSUM. Tolerance is
2e-2 and the bf16 pipeline measures ~3e-3 max-rel in simulation.

Per-core pipeline (heads processed in pairs m; hp = head parity):
  V-proj   v[t, f] staged per pair as [128, pair, hp, 128] bf16 with an
           ones column (hp0 at col 64, hp1 at col 63 with zero pad below
           and v at cols 64:128) so AV emits the softmax denominator and
           the odd head's output lands partition-aligned at rows 64:128.
  per pair m, units (jb, ih) over key-block x query-half:
    QK^T   both heads concurrently in disjoint PE row groups into one
           [128, 2, 512] PSUM tile (hp0 bank 0, hp1 bank 1), causal-
           trimmed to columns off..512 where off = jb*128 - ih*512.
    mask   one strided DVE add of -32768 on the diagonal block.
    exp    one strided ACT instruction for both heads -> pt bf16.
    AV     accumulate ya[hp] over jb per query-half (ones col -> denom).
    kqproj for pair m+1 interleaved between units as PE filler.
  normalize: ACT evicts ya->yr, DVE reciprocal of denom rows, DRAM-hop
           partition-broadcast, one DVE multiply per head -> yt bf16.
  out-proj per 128-row block: 8 pair matmuls + K=1 ones-row matmul that
           adds the bias, evict fp32, DMA out.
"""
import sys
from contextlib import ExitStack

sys.path.insert(0, "/opt/trn_rl_repo")
import numpy as np
import ml_dtypes

from concourse import bacc, mybir
from concourse import tile
from concourse.bass_utils import run_bass_kernel_spmd

B, T, C = 8, 1024, 1024
H = 16
D = C // H  # 64
NCORES = 8
NPAIR = H // 2  # 8
NTB = T // 128  # 8
NCB = C // 128  # 8
F32 = mybir.dt.float32
BF16 = mybir.dt.bfloat16
AF = mybir.ActivationFunctionType
SCALE = 1.0 / 8.0  # 1/sqrt(D)
NEG = -32768.0

# units per pair: (jb, ih, off, diag)
UNITS = []
for ih in range(2):
    for jb in range(4 * (ih + 1)):
        off = max(0, jb * 128 - ih * 512)
        UNITS.append((jb, ih, off, jb // 4 == ih))


def build():
    nc = bacc.Bacc(target_bir_lowering=False)
    xT = nc.dram_tensor("xT", [C, T], BF16, kind="ExternalInput")
    wqk = nc.dram_tensor("wqk", [C, NPAIR, 256], BF16, kind="ExternalInput")
    wvT = nc.dram_tensor("wvT", [C, C], BF16, kind="ExternalInput")
    wpT = nc.dram_tensor("wpT", [C, C], BF16, kind="ExternalInput")
    bb = nc.dram_tensor("bb", [128, C], F32, kind="ExternalInput")
    identb = nc.dram_tensor("identb", [128, 128], BF16, kind="ExternalInput")
    maskb = nc.dram_tensor("maskb", [128, 128], BF16, kind="ExternalInput")
    out = nc.dram_tensor("out", [T, C], F32, kind="ExternalOutput")

    with tile.TileContext(nc) as tc, ExitStack() as top:
        const = top.enter_context(tc.tile_pool(name="const", bufs=1))
        ytp = top.enter_context(tc.tile_pool(name="yt", bufs=1))
        psa = top.enter_context(tc.tile_pool(name="psa", bufs=1, space="PSUM"))
        psatt = top.enter_context(tc.tile_pool(name="psatt", bufs=1,
                                               space="PSUM"))
        psya = top.enter_context(tc.tile_pool(name="psya", bufs=1,
                                              space="PSUM"))
        dramp = top.enter_context(tc.tile_pool(name="dram", bufs=1,
                                               space="DRAM"))
        xp = top.enter_context(tc.tile_pool(name="xp", bufs=1))
        vtp = top.enter_context(tc.tile_pool(name="vt", bufs=1))
        wq = top.enter_context(tc.tile_pool(name="wq", bufs=1))
        kqp = top.enter_context(tc.tile_pool(name="kq", bufs=1))
        ptp = top.enter_context(tc.tile_pool(name="pt", bufs=1))
        yrp = top.enter_context(tc.tile_pool(name="yr", bufs=1))
        recp = top.enter_context(tc.tile_pool(name="rec", bufs=1))
        bcp = top.enter_context(tc.tile_pool(name="bc", bufs=1))
        osp = top.enter_context(tc.tile_pool(name="os", bufs=1))

        # constants
        pstat = const.tile([128, 128], BF16, name="pstat")
        nc.scalar.dma_start(out=pstat[:], in_=xT[0:128, 0:128])
        pmov = const.tile([128, 512], BF16, name="pmov")
        nc.scalar.dma_start(out=pmov[:], in_=xT[0:128, 0:512])
        idt = const.tile([128, 128], BF16, name="idt")
        nc.sync.dma_start(out=idt[:], in_=identb[:])
        mkt = const.tile([128, 128], BF16, name="mkt")
        nc.sync.dma_start(out=mkt[:], in_=maskb[:])
        bbt = const.tile([128, C], F32, name="bbt")
        nc.sync.dma_start(out=bbt[:], in_=bb[:])

        # warm-up primer: keep the PE busy while the x/w DMAs land.
        prim = psa.tile([128, 512], F32, name="prim", tag="a", bufs=1)
        for _ in range(24):
            nc.tensor.matmul(prim[:], pstat[:], pmov[:], start=True,
                             stop=True)

        # ---- x load ----
        xts = []
        for cb in range(NCB):
            xt = xp.tile([128, T], BF16, name=f"x{cb}", tag="x", bufs=NCB)
            nc.scalar.dma_start(out=xt[:],
                                in_=xT[cb * 128:(cb + 1) * 128, :])
            xts.append(xt)

        # ---- attention ----
        wqms = {}

        def load_wqm(mm_):
            wqm_ = wq.tile([128, NCB, 256], BF16, name=f"wqm{mm_}",
                           tag="wqm", bufs=2)
            nc.sync.dma_start(
                out=wqm_[:],
                in_=wqk[:, mm_, :].rearrange("(cb p) f -> p cb f", p=128))
            wqms[mm_] = wqm_

        kqs = {}
        kq_chunks = [(0, 0), (0, 1), (1, 0), (1, 1)]
        kq_progress = {}

        def kqproj_group(mm_, kq, th):
            """One accumulation group of the K/Q projection for pair mm_."""
            if mm_ not in kqs:
                km_ = kqp.tile([128, T], BF16, name=f"k{mm_}", tag="km",
                               bufs=2)
                qm_ = kqp.tile([128, T], BF16, name=f"q{mm_}", tag="qm",
                               bufs=2)
                kqs[mm_] = (km_, qm_)
            wqm_ = wqms[mm_]
            dst = kqs[mm_][kq]
            ps = psa.tile([128, 512], F32, name="kqps", tag="a", bufs=1)
            for cb in range(NCB):
                nc.tensor.matmul(
                    ps[:], wqm_[:, cb, kq * 128:(kq + 1) * 128],
                    xts[cb][:, th * 512:(th + 1) * 512],
                    start=(cb == 0), stop=(cb == NCB - 1))
            nc.vector.tensor_copy(dst[:, th * 512:(th + 1) * 512], ps[:])

        def next_kq_chunk(mm_):
            if mm_ >= NPAIR:
                return
            i = kq_progress.get(mm_, 0)
            if i < 4:
                kq_progress[mm_] = i + 1
                kqproj_group(mm_, *kq_chunks[i])

        load_wqm(0)
        load_wqm(1)

        # ---- V projection into per-pair aug layout ----
        vts = []
        with tc.tile_pool(name="wv", bufs=1) as wv:
            wvt = wv.tile([128, NCB, C], BF16, name="wvt")
            for cb in range(NCB):
                nc.sync.dma_start(out=wvt[:, cb, :],
                                  in_=wvT[cb * 128:(cb + 1) * 128, :])
            for tb in range(NTB):
                vt = vtp.tile([128, NPAIR, 2, 128], BF16, name=f"v{tb}",
                              tag="v", bufs=NTB)
                # zero pad below hp1's v, ones cols for the denominators
                # (hp1 ones at col 32 -> denom lands at psum partition 32,
                #  a 32-aligned partition base)
                nc.gpsimd.memset(vt[:, :, 1, 0:64], 0.0)
                nc.gpsimd.memset(vt[:, :, 1, 32:33], 1.0)
                nc.gpsimd.memset(vt[:, :, 0, 64:128], 0.0)
                nc.gpsimd.memset(vt[:, :, 0, 64:65], 1.0)
                for half in range(2):
                    ps = psatt.tile([128, 1024], F32, name="vps", tag="att",
                                    bufs=2)
                    for cb in range(NCB):
                        nc.tensor.matmul(
                            ps[:, 0:512],
                            xts[cb][:, tb * 128:(tb + 1) * 128],
                            wvt[:, cb, half * 512:(half + 1) * 512],
                            start=(cb == 0), stop=(cb == NCB - 1))
                    psv = ps[:, 0:512].rearrange("p (pr hp f) -> p pr hp f",
                                                 hp=2, f=64)
                    eng = nc.vector if half == 0 else nc.scalar
                    if half == 0:
                        nc.vector.tensor_copy(
                            vt[:, 0:4, 0, 0:64], psv[:, :, 0, :])
                        nc.vector.tensor_copy(
                            vt[:, 0:4, 1, 64:128], psv[:, :, 1, :])
                    else:
                        nc.scalar.activation(
                            vt[:, 4:8, 0, 0:64], psv[:, :, 0, :], AF.Copy)
                        nc.scalar.activation(
                            vt[:, 4:8, 1, 64:128], psv[:, :, 1, :], AF.Copy)
                vts.append(vt)
                if tb >= 1:
                    next_kq_chunk(0 if tb <= 4 else 1)


        # output-projection weights: load early so out-proj never waits
        wp = top.enter_context(tc.tile_pool(name="wp", bufs=1))
        wpt = wp.tile([128, NCB, C], BF16, name="wpt")
        nc.scalar.dma_start(
            out=wpt[:], in_=wpT[:].rearrange("(cb p) j -> p cb j", p=128))

        yts = []
        pending_mul = [None]
        for m in range(NPAIR):
            km, qm = kqs[m]
            yt2 = [ytp.tile([128, 512], BF16, name=f"yt{m}_{ih}",
                            tag=f"yt{ih}", bufs=NPAIR) for ih in range(2)]
            yts.append(yt2)

            ya = {}   # (ih, hp) -> psum tile
            pts = {}  # unit index -> pt tile

            def av_unit(u):
                jb, ih, off, diag = UNITS[u]
                w = 512 - off
                pt = pts[u]
                jmax = 4 * (ih + 1) - 1
                for hp in range(2):
                    if jb == 0:
                        ya[(ih, hp)] = psya.tile(
                            [128, 512], F32, name=f"ya{m}_{ih}_{hp}",
                            tag="ya", bufs=3)
                    dst = ya[(ih, hp)]
                    if hp == 0:
                        nc.tensor.matmul(
                            dst[:, off:512], vts[jb][:, m, 0, :],
                            pt[:, 0, :], start=(jb == 0), stop=(jb == jmax))
                    else:
                        nc.tensor.matmul(
                            dst[:, off:512], vts[jb][:, m, 1, :],
                            pt[:, 1, :], start=(jb == 0), stop=(jb == jmax))

            yr0 = yrp.tile([128, T], F32, name=f"yr0_{m}", tag="yr0", bufs=2)
            yr1 = yrp.tile([128, T], F32, name=f"yr1_{m}", tag="yr1", bufs=2)

            rec = recp.tile([65, T], F32, name=f"rec{m}", tag="rec", bufs=2)
            dd = dramp.tile([2, T], F32, name=f"dd{m}", tag="dd", bufs=2)
            dtr = recp.tile([128, 2, 8], F32, name=f"dtr{m}", tag="dtr",
                            bufs=2)
            rtr = recp.tile([128, 2, 8], F32, name=f"rtr{m}", tag="rtr",
                            bufs=2)
            db = dramp.tile([2, T], F32, name=f"db{m}", tag="db", bufs=2)
            bc = bcp.tile([128, T], F32, name=f"bc{m}", tag="bc", bufs=2)

            def evict_ih(ih):
                c0, c1 = ih * 512, ih * 512 + 512
                p0 = ih * 64
                nc.vector.tensor_copy(yr0[0:65, c0:c1],
                                      ya[(ih, 0)][0:65, :])
                nc.vector.tensor_copy(yr1[64:128, c0:c1],
                                      ya[(ih, 1)][64:128, :])
                # hp1 denominator to SBUF (psum is not DMA-able)
                nc.vector.tensor_copy(rec[32:33, c0:c1],
                                      ya[(ih, 1)][32:33, :])
                # reshape this half's denoms to [64, 2, 8], cheap reciprocal,
                # broadcast 1/d along partitions via DRAM hop
                nc.sync.dma_start(out=dd[0:1, c0:c1], in_=rec[32:33, c0:c1])
                nc.sync.dma_start(out=dd[1:2, c0:c1], in_=yr0[64:65, c0:c1])
                nc.sync.dma_start(
                    out=dtr[p0:p0 + 64, :, :],
                    in_=dd[:, c0:c1].rearrange("two (p q) -> p two q", q=8))
                nc.vector.reciprocal(rtr[p0:p0 + 64, :, :],
                                     dtr[p0:p0 + 64, :, :])
                nc.sync.dma_start(
                    out=db[:, c0:c1].rearrange("two (p q) -> p two q", q=8),
                    in_=rtr[p0:p0 + 64, :, :])
                nc.sync.dma_start(out=bc[0:64, c0:c1],
                                  in_=db[1, c0:c1].partition_broadcast(64))
                nc.sync.dma_start(out=bc[64:128, c0:c1],
                                  in_=db[0, c0:c1].partition_broadcast(64))

            def mul_ih(ih, yt2=yt2, yr0=yr0, yr1=yr1, bc=bc):
                c0, c1 = ih * 512, ih * 512 + 512
                nc.vector.tensor_mul(yt2[ih][0:64, :], yr0[0:64, c0:c1],
                                     bc[0:64, c0:c1])
                nc.vector.tensor_mul(yt2[ih][64:128, :], yr1[64:128, c0:c1],
                                     bc[64:128, c0:c1])

            for u, (jb, ih, off, diag) in enumerate(UNITS):
                w = 512 - off
                att = psatt.tile([128, 2, 512], F32, name=f"att{u}",
                                 tag="att", bufs=2)
                for hp in range(2):
                    p0 = hp * 64
                    nc.tensor.matmul(
                        att[:, hp, off:512],
                        km[p0:p0 + 64, jb * 128:(jb + 1) * 128],
                        qm[p0:p0 + 64, ih * 512 + off:ih * 512 + 512],
                        start=True, stop=not diag)
                if diag:
                    # causal mask folded into the accumulation group:
                    # att[:, hp, diag] += I.T @ mask
                    for hp in range(2):
                        nc.tensor.matmul(att[:, hp, off:off + 128],
                                         idt[:], mkt[:],
                                         start=False, stop=True)
                pt = ptp.tile([128, 2, w], BF16, name=f"pt{u}", tag=f"pt{u}",
                              bufs=2)
                nc.scalar.activation(pt[:], att[:, :, off:512], AF.Exp,
                                     scale=SCALE)
                pts[u] = pt
                if u > 0:
                    av_unit(u - 1)
                    if UNITS[u - 1][1] == 0 and UNITS[u - 1][0] == 3:
                        evict_ih(0)
                if u == 6 and pending_mul[0] is not None:
                    pending_mul[0](1)  # prev pair's ih1 normalize multiply
                    pending_mul[0] = None
                if m + 1 < NPAIR and u in (2, 5, 8, 10):
                    next_kq_chunk(m + 1)
            av_unit(len(UNITS) - 1)
            evict_ih(1)
            mul_ih(0)
            if m + 1 < NPAIR:
                pending_mul[0] = mul_ih
            else:
                pending_mul[0] = None
                last_mul1 = mul_ih
            if m + 2 < NPAIR:
                load_wqm(m + 2)

        # ---- output projection ----
        # tb 0..3 read only the ih0 halves of yt, so they start while the
        # last pair's ih1 normalize chain is still in flight
        for tb in range(NTB):
            if tb == 2:
                last_mul1(1)
            ih = tb // 4
            lc = tb * 128 - ih * 512
            ps = psatt.tile([128, 1024], F32, name="ops", tag="att",
                            bufs=2)
            for half in range(2):
                dst = ps[:, half * 512:(half + 1) * 512]
                for mm in range(NPAIR):
                    nc.tensor.matmul(
                        dst, yts[mm][ih][:, lc:lc + 128],
                        wpt[:, mm, half * 512:(half + 1) * 512],
                        start=(mm == 0), stop=(mm == NPAIR - 1))
            ost = osp.tile([128, C], F32, name="ost", tag="ost", bufs=2)
            nc.vector.tensor_add(ost[:, 0:512], ps[:, 0:512],
                                 bbt[:, 0:512])
            nc.vector.tensor_add(ost[:, 512:1024], ps[:, 512:1024],
                                 bbt[:, 512:1024])
            nc.sync.dma_start(out=out[tb * 128:(tb + 1) * 128, :],
                              in_=ost[:])

    nc.compile()
    return nc


_NC = None


def _get_nc():
    global _NC
    if _NC is None:
        _NC = build()
    return _NC


def prep_inputs(x, w_attn, w_proj, b_proj):
    BF = ml_dtypes.bfloat16
    x = np.asarray(x, dtype=np.float32)
    w_attn = np.asarray(w_attn, dtype=np.float32)
    w_proj = np.asarray(w_proj, dtype=np.float32)
    b_proj = np.asarray(b_proj, dtype=np.float32)
    ki = np.ascontiguousarray(w_attn[0:C].T).reshape(C, NPAIR, 128)
    qi = np.ascontiguousarray(w_attn[C:2 * C].T).reshape(C, NPAIR, 128)
    wqkv = np.ascontiguousarray(np.concatenate([ki, qi], axis=2)).astype(BF)
    wvTv = np.ascontiguousarray(w_attn[2 * C:3 * C].T).astype(BF)
    wpTv = np.ascontiguousarray(w_proj.T).astype(BF)
    bbv = np.broadcast_to(b_proj, (128, C)).copy()
    ii = np.arange(128)
    mk = np.where(ii[None, :] >= ii[:, None], 0.0, NEG).astype(BF)
    idn = np.eye(128, dtype=BF)
    shared = {"wqk": wqkv, "wvT": wvTv, "wpT": wpTv, "bb": bbv,
              "maskb": mk, "identb": idn}
    in_maps = []
    for b in range(B):
        im = dict(shared)
        im["xT"] = np.ascontiguousarray(x[b].T).astype(BF)
        in_maps.append(im)
    return in_maps


def kernel(x, w_attn, w_proj, b_proj):
    nc = _get_nc()
    in_maps = prep_inputs(x, w_attn, w_proj, b_proj)
    res = run_bass_kernel_spmd(nc, in_maps, core_ids=list(range(NCORES)))
    return np.stack([res.results[b]["out"]
                     for b in range(B)]).astype(np.float32)
